# revision 7
# baseline (speedup 1.0000x reference)
"""Compressed-KV GPT-2 attention block on 8 TRN2 NeuronCores.

Sharding: batch x head-group. Core c: batch b = c//4, heads 4*(c%4)..4*(c%4)+4.

Structure (v3): attention runs in the compressed C-space (C=32 < hd=64);
all decompressors fold into host-side weights, and the v-compressor folds
into the qkv projection:
  w_q' = w_q wk_d^T/8 [D,32/head]   w_k' = w_k wk_c [D,32/head]
  w_v' = w_v wv_c    [D,32/head]    w_p' = wv_d_h w_proj_h [32,D]
Device pipeline per core (bf16 matmuls -> fp32 PSUM):
  qkv^T    = w' chunks @ hidden^T      (3 m-blocks: q'(4hx32) | kc | vc)
  vco      = PE-transpose(vc^T k-tile) (-> [kpos, 4x(32|ones)] SBUF)
  S^T      = kc-slice^T @ q'  (K=32 per head) -> batched exp (Scalar)
             -> diagonal band tiles masked on DVE
  attn[q,c]= E-tile^T(stationary) @ vco-slice  (M=128 queries, N=33:
             col 32 = ones = softmax denom per query partition)
  norm     = DVE strided recip + broadcast mult -> [q, 4hx32] bf16
  attn^T   = PE-transpose -> c_proj: one K=128 contraction over all heads
  out[q,d] accumulated per 128-query block, DMA'd to out_s [S, D]
"""

import sys

if "/opt/trn_rl_repo" not in sys.path:
    sys.path.insert(0, "/opt/trn_rl_repo")

import numpy as np
import ml_dtypes

BF16 = ml_dtypes.bfloat16

B, S, D = 2, 2048, 1024
H, hd, C = 16, 64, 32
NCORES = 8
HPC = 4            # heads per core
SB = 512           # q block per score tile / PSUM bank cols
NSB = S // SB      # 4 seq blocks of 512
NKT = S // 128     # 16 key tiles of 128
DC = D // 128      # 8 contraction chunks for qkv
VW = C + 1         # attn + den column width per head slot

_cache = {}


def _build():
    import concourse.bacc as bacc
    import concourse.tile as tile
    import concourse.mybir as mybir

    dt = mybir.dt
    f32, bf16 = dt.float32, dt.bfloat16
    Exp = mybir.ActivationFunctionType.Exp
    mult = mybir.AluOpType.mult

    nc = bacc.Bacc("TRN2", target_bir_lowering=False, debug=False, num_devices=NCORES)

    hidden_t = nc.dram_tensor("hidden_t", [D, S], bf16, kind="ExternalInput")
    w_qkv = nc.dram_tensor("w_qkv", [D, 3 * 128], bf16, kind="ExternalInput")
    b_qkv = nc.dram_tensor("b_qkv", [128, 3], f32, kind="ExternalInput")
    wpj = nc.dram_tensor("wpj", [128, D], bf16, kind="ExternalInput")
    maskd = nc.dram_tensor("maskd", [128, 128], bf16, kind="ExternalInput")
    idend = nc.dram_tensor("idend", [128, 128], bf16, kind="ExternalInput")
    out_s = nc.dram_tensor("out_s", [S, D], bf16, kind="ExternalOutput")

    with tile.TileContext(nc) as tc:
        with (
            tc.tile_pool(name="persist", bufs=1) as pp,
            tc.tile_pool(name="epool", bufs=12) as ep,
            tc.tile_pool(name="apool", bufs=3) as apo,
            tc.tile_pool(name="tpool", bufs=3) as tpo,
            tc.tile_pool(name="opool", bufs=4) as op,
            tc.tile_pool(name="rpool", bufs=3) as rp,
            tc.tile_pool(name="ps_sc", bufs=1, space="PSUM") as ps_sc,
            tc.tile_pool(name="ps_pv", bufs=2, space="PSUM") as ps_pv,
            tc.tile_pool(name="ps_cp", bufs=1, space="PSUM") as ps_cp,
            tc.tile_pool(name="ps_tr", bufs=1, space="PSUM") as ps_tr,
        ):
            # ---- small weights first, then hidden sb0 ----
            bias = pp.tile([128, 3], f32, tag="bias", name="bias")
            nc.sync.dma_start(bias[:], b_qkv.ap())
            wq = []
            for d in range(DC):
                w = pp.tile([128, 3 * 128], bf16, tag=f"wq{d}", name=f"wq{d}")
                nc.sync.dma_start(w[:], w_qkv.ap()[d * 128:(d + 1) * 128, :])
                wq.append(w)
            wpj_t = pp.tile([128, D], bf16, tag="wpj", name="wpj")
            nc.sync.dma_start(wpj_t[:], wpj.ap())
            maskt = pp.tile([128, 128], bf16, tag="mask", name="maskt")
            nc.sync.dma_start(maskt[:], maskd.ap())
            ident = pp.tile([128, 128], bf16, tag="ident", name="ident")
            nc.sync.dma_start(ident[:], idend.ap())

            hT = [pp.tile([128, S], bf16, tag=f"hT{d}", name=f"hT{d}") for d in range(DC)]
            sl0 = slice(0, SB)
            for d in range(DC):
                nc.sync.dma_start(hT[d][:, sl0], hidden_t.ap()[d * 128:(d + 1) * 128, sl0])

            # qkv m-block destinations (transposed layouts, [rows, seq])
            qcT = pp.tile([128, S], bf16, tag="qcT", name="qcT")
            kcT = pp.tile([128, S], bf16, tag="kcT", name="kcT")
            vcT = pp.tile([128, S], bf16, tag="vcT", name="vcT")
            # head 3 lives at partition 96; PE wants operand base in {0,32,64}
            qc3 = pp.tile([64, S], bf16, tag="qc3", name="qc3")
            kc3 = pp.tile([64, S], bf16, tag="kc3", name="kc3")

            # v_comp per k-tile: [kpos 128, 4 x (32 attn | 1 ones)]
            vco = pp.tile([128, NKT * HPC * VW], bf16, tag="vco", name="vco")
            nc.vector.memset(
                vco[:].rearrange("p (t w) -> p t w", w=VW)[:, :, C:C + 1], 1.0
            )

            def emit_qkv(sb):
                sl = slice(sb * SB, (sb + 1) * SB)
                if sb > 0:
                    for d in range(DC):
                        nc.sync.dma_start(
                            hT[d][:, sl],
                            hidden_t.ap()[d * 128:(d + 1) * 128, sl],
                        )
                psA = ps_sc.tile([128, 2 * SB], f32, tag="psS", name="psA")
                for half, mb in ((0, 0), (1, 1)):
                    for d in range(DC):
                        nc.tensor.matmul(
                            psA[:, half * SB:(half + 1) * SB],
                            wq[d][:, mb * 128:(mb + 1) * 128],
                            hT[d][:, sl],
                            start=(d == 0),
                            stop=(d == DC - 1),
                        )
                psB = ps_sc.tile([128, 2 * SB], f32, tag="psS", name="psB")
                for d in range(DC):
                    nc.tensor.matmul(
                        psB[:, 0:SB],
                        wq[d][:, 2 * 128:3 * 128],
                        hT[d][:, sl],
                        start=(d == 0),
                        stop=(d == DC - 1),
                    )
                nc.vector.tensor_scalar_add(
                    out=qcT[:, sl], in0=psA[:, 0:SB], scalar1=bias[:, 0:1]
                )
                nc.vector.tensor_scalar_add(
                    out=kcT[:, sl], in0=psA[:, SB:2 * SB], scalar1=bias[:, 1:2]
                )
                nc.vector.tensor_scalar_add(
                    out=vcT[:, sl], in0=psB[:, 0:SB], scalar1=bias[:, 2:3]
                )
                nc.sync.dma_start(qc3[32:64, sl], qcT[96:128, sl])
                nc.sync.dma_start(kc3[32:64, sl], kcT[96:128, sl])
                # v transpose: [c-rows, k] -> [k, c] per k-tile, into vco slots
                for kt in range(4 * sb, 4 * sb + 4):
                    pst = ps_tr.tile([128, 128], bf16, tag="psT", name="psT")
                    nc.tensor.transpose(
                        pst[:], vcT[:, kt * 128:(kt + 1) * 128], ident[:]
                    )
                    nc.vector.tensor_copy(
                        vco[:, kt * HPC * VW:(kt + 1) * HPC * VW]
                        .rearrange("p (h w) -> p h w", h=HPC)[:, :, 0:C],
                        pst[:].rearrange("p (h c) -> p h c", h=HPC),
                    )

            # per-sb shared PV psum tiles: [q 128, 2 qb x (4h x 33)]
            def emit_attention(sb):
                nkb = 4 * sb + 4
                pvt = [
                    ps_pv.tile([128, HPC * VW], f32, tag="psP", name="psP", bufs=4)
                    for _ in range(4)
                ]
                e_of = {}       # kb -> (tile, col_off, c0)
                att = {}        # qb -> attn_all sbuf tile

                def pv_col(qb, h):
                    return h * VW

                def emit_pv(h, kb):
                    e, off, c0 = e_of[kb]
                    for qb in range(max(kb, 4 * sb), 4 * sb + 4):
                        qo = (qb % 4) * 128
                        if qo < c0:
                            continue
                        nc.tensor.matmul(
                            pvt[qb % 4][:, pv_col(qb, h):pv_col(qb, h) + VW],
                            e[:, off + qo:off + qo + 128],
                            vco[:, kb * HPC * VW + h * VW:kb * HPC * VW + (h + 1) * VW],
                            start=(kb == 0),
                            stop=(kb == qb),
                        )

                def emit_scores(h, batch):
                    if h == 3:
                        ksrc, qsrc, hsl = kc3, qc3, slice(32, 64)
                    else:
                        ksrc, qsrc, hsl = kcT, qcT, slice(32 * h, 32 * h + 32)
                    psS = ps_sc.tile([128, 2 * SB], f32, tag="psS", name="psS")
                    e = ep.tile([128, 2 * SB], bf16, tag="E", name="e")
                    first_c0 = None
                    for i, kb in enumerate(batch):
                        r = kb - 4 * sb
                        c0 = max(r, 0) * 128
                        if first_c0 is None:
                            first_c0 = c0
                        nc.tensor.matmul(
                            psS[:, i * SB + c0:(i + 1) * SB],
                            ksrc[hsl, kb * 128:(kb + 1) * 128],
                            qsrc[hsl, sb * SB + c0:(sb + 1) * SB],
                        )
                        e_of[kb] = (e, i * SB, c0)
                    last = len(batch) * SB
                    nc.scalar.activation(
                        e[:, first_c0:last], psS[:, first_c0:last], Exp
                    )
                    for i, kb in enumerate(batch):
                        r = kb - 4 * sb
                        if r >= 0:
                            c0 = r * 128
                            nc.gpsimd.tensor_tensor(
                                e[:, i * SB + c0:i * SB + c0 + 128],
                                e[:, i * SB + c0:i * SB + c0 + 128],
                                maskt[:], mult,
                            )

                def emit_norm(qb):
                    half = pvt[qb % 4]
                    off = 0
                    rc = rp.tile([128, HPC], f32, tag="rc", name="rc")
                    nc.vector.reciprocal(
                        rc[:].unsqueeze(2),
                        half[:, off:off + HPC * VW]
                        .rearrange("p (h w) -> p h w", h=HPC)[:, :, C:C + 1],
                    )
                    a = apo.tile([128, 128], bf16, tag="attn", name="attn")
                    att[qb] = a
                    nc.vector.tensor_tensor(
                        a[:].rearrange("p (h c) -> p h c", h=HPC),
                        half[:, off:off + HPC * VW]
                        .rearrange("p (h w) -> p h w", h=HPC)[:, :, 0:C],
                        rc[:].unsqueeze(2).to_broadcast([128, HPC, C]),
                        mult,
                    )

                def emit_cproj(qb):
                    a = att.pop(qb)
                    pst = ps_tr.tile([128, 128], bf16, tag="psT", name="psT")
                    nc.tensor.transpose(pst[:], a[:], ident[:])
                    aT = tpo.tile([128, 128], bf16, tag="attnT", name="attnT")
                    nc.vector.tensor_copy(aT[:], pst[:])
                    for half in range(2):
                        pso = ps_cp.tile([128, SB], f32, tag="psO", name="psO")
                        nc.tensor.matmul(
                            pso[:],
                            aT[:],
                            wpj_t[:, half * SB:(half + 1) * SB],
                        )
                        stage = op.tile([128, SB], bf16, tag="stage", name="stage")
                        nc.vector.tensor_copy(stage[:], pso[:])
                        nc.sync.dma_start(
                            out_s.ap()[qb * 128:(qb + 1) * 128,
                                       half * SB:(half + 1) * SB],
                            stage[:],
                        )

                for h in range(HPC):
                    batches = [
                        list(range(bb, min(bb + 2, nkb))) for bb in range(0, nkb, 2)
                    ]
                    prev = None
                    for batch in batches:
                        emit_scores(h, batch)
                        if prev is not None:
                            for kb in prev:
                                emit_pv(h, kb)
                        prev = batch
                    for kb in prev:
                        emit_pv(h, kb)
                    if h == 3:
                        for qb in range(4 * sb, 4 * sb + 4):
                            emit_norm(qb)
                        for qb in range(4 * sb, 4 * sb + 4):
                            emit_cproj(qb)

            for sb in range(NSB):
                emit_qkv(sb)
                emit_attention(sb)

    nc.compile()
    return nc


def _prep_inputs(hidden_states, w_attn, b_attn, wk_c, wv_c, wk_d, wv_d, w_proj):
    """Per-core input maps: fold all compressors/decompressors into weights.

      w_q' = w_q @ wk_d^T / sqrt(hd)   [D, C]  (scores contract over C)
      w_k' = w_k @ wk_c                [D, C]
      w_v' = w_v @ wv_c                [D, C]  (v-compress folded into qkv)
      w_p'_h = wv_d_h @ w_proj_h       [C, D]  (c_proj contracts over 4h x C)
    """
    f8 = np.float64
    hidden_T = [np.ascontiguousarray(hidden_states[b].T).astype(BF16) for b in range(B)]
    scale = 1.0 / np.sqrt(hd)
    wq_h = lambda h: (w_attn[:, h * hd:(h + 1) * hd].astype(f8)
                      @ wk_d[h].astype(f8).T * scale).astype(np.float32)
    bq_h = lambda h: (b_attn[h * hd:(h + 1) * hd].astype(f8)
                      @ wk_d[h].astype(f8).T * scale).astype(np.float32)
    wk_h = lambda h: (w_attn[:, D + h * hd:D + (h + 1) * hd].astype(f8)
                      @ wk_c[h].astype(f8)).astype(np.float32)
    bk_h = lambda h: (b_attn[D + h * hd:D + (h + 1) * hd].astype(f8)
                      @ wk_c[h].astype(f8)).astype(np.float32)
    wv_h = lambda h: (w_attn[:, 2 * D + h * hd:2 * D + (h + 1) * hd].astype(f8)
                      @ wv_c[h].astype(f8)).astype(np.float32)
    bv_h = lambda h: (b_attn[2 * D + h * hd:2 * D + (h + 1) * hd].astype(f8)
                      @ wv_c[h].astype(f8)).astype(np.float32)
    wpj_h = lambda h: (wv_d[h].astype(f8)
                       @ w_proj[h * hd:(h + 1) * hd, :].astype(f8)).astype(np.float32)

    k = np.arange(128).reshape(128, 1)
    j = np.arange(128).reshape(1, 128)
    mask = (k <= j).astype(BF16)
    iden = np.eye(128, dtype=BF16)

    in_maps = []
    for c in range(NCORES):
        b = c // 4
        hs = list(range((c % 4) * HPC, (c % 4) * HPC + HPC))
        # m-blocks: [q'0..3], [kc0..3], [vc0..3]
        cols = ([wq_h(h) for h in hs] + [wk_h(h) for h in hs]
                + [wv_h(h) for h in hs])
        w_qkv_l = np.concatenate(cols, axis=1).astype(BF16)        # [1024, 384]
        bcols = ([bq_h(h) for h in hs] + [bk_h(h) for h in hs]
                 + [bv_h(h) for h in hs])
        b_qkv_l = (np.concatenate(bcols).astype(np.float32)
                   .reshape(3, 128).T.copy())                      # [128, 3]
        # w_proj' stacked over the core's 4 heads: [4h x 32, D]
        wpj_l = np.concatenate([wpj_h(h) for h in hs], axis=0).astype(BF16)
        in_maps.append(
            {
                "hidden_t": hidden_T[b],
                "w_qkv": w_qkv_l,
                "b_qkv": b_qkv_l,
                "wpj": wpj_l,
                "maskd": np.ascontiguousarray(mask),
                "idend": iden,
            }
        )
    return in_maps


def kernel(
    hidden_states,
    w_attn,
    b_attn,
    w_proj,
    b_proj,
    wk_c,
    wv_c,
    wk_d,
    wv_d,
    _trace=False,
):
    from concourse.bass_utils import run_bass_kernel_spmd

    if "nc" not in _cache:
        _cache["nc"] = _build()
    nc = _cache["nc"]

    in_maps = _prep_inputs(
        np.asarray(hidden_states),
        np.asarray(w_attn),
        np.asarray(b_attn),
        np.asarray(wk_c),
        np.asarray(wv_c),
        np.asarray(wk_d),
        np.asarray(wv_d),
        np.asarray(w_proj),
    )
    res = run_bass_kernel_spmd(
        nc, in_maps, core_ids=list(range(NCORES)), trace=_trace
    )
    out = np.empty((B, S, D), np.float32)
    for b in range(B):
        acc = np.zeros((S, D), np.float32)
        for c in range(4 * b, 4 * b + 4):
            acc += res.results[c]["out_s"].astype(np.float32)
        out[b] = acc + np.asarray(b_proj, np.float32)
    if _trace:
        _cache["last_exec_time_ns"] = res.exec_time_ns
        _cache["last_results"] = res
    return out


# revision 12
# speedup vs baseline: 1.3157x; 1.3157x over previous
"""Compressed-KV GPT-2 attention block on 8 TRN2 NeuronCores.

Sharding: batch x head-group. Core c: batch b = c//4, heads 4*(c%4)..4*(c%4)+4.

Structure (v4): attention runs in the compressed C-space (C=32 < hd=64);
all decompressors fold into host-side weights, and the v-compressor folds
into the qkv projection:
  w_q' = w_q wk_d^T/8 [D,32/head]   w_k' = w_k wk_c [D,32/head]
  w_v' = w_v wv_c    [D,32/head]    w_p' = wv_d_h w_proj_h [32,D]
Device pipeline per core (bf16 matmuls -> fp32 PSUM):
  qkv^T    = w' chunks @ hidden^T      (3 m-blocks: q'(4hx32) | kc | vc)
  vco      = DMA-transpose(vc^T k-tile) -> [kpos, 4x(32|ones)] SBUF
  S^T      = kc-slice^T @ q'  (K=32 per head) -> 2-bank batched exp (Scalar)
             -> diagonal band tiles masked on GpSimd
  attn[q,c]= E-tile^T(stationary) @ vco-slice, per-(head,qb) bursts so each
             PSUM bank has one open accumulation group (M=128 queries, N=33:
             col 32 = ones = softmax denom per query partition)
  norm     = one DVE divide per (head, qb): psum/den -> attn_all bf16
  attn^T   = DMA-transpose -> c_proj: one K=128 contraction over all heads
  out[q,d] 2 x 512-col matmuls per 128-query block, DMA'd to out_s [S, D]
Schedule: PV bursts of head h-1 interleave into head h's score batches;
qkv(sb+1) m-block chunks interleave into head 3's batches; the previous
sb's PV(h3)+cproj interleave into the next sb's head-0 batches. This keeps
the Scalar engine (exp, the ~73us roofline here) continuously fed.
"""

import sys

if "/opt/trn_rl_repo" not in sys.path:
    sys.path.insert(0, "/opt/trn_rl_repo")

import numpy as np
import ml_dtypes

BF16 = ml_dtypes.bfloat16

B, S, D = 2, 2048, 1024
H, hd, C = 16, 64, 32
NCORES = 8
HPC = 4            # heads per core
SB = 512           # q block per score tile / PSUM bank cols
NSB = S // SB      # 4 seq blocks of 512
NKT = S // 128     # 16 key tiles of 128
DC = D // 128      # 8 contraction chunks for qkv
VW = C + 1         # attn + den column width per head slot

_cache = {}


def _build():
    import concourse.bacc as bacc
    import concourse.tile as tile
    import concourse.mybir as mybir

    dt = mybir.dt
    f32, bf16 = dt.float32, dt.bfloat16
    Exp = mybir.ActivationFunctionType.Exp
    mult = mybir.AluOpType.mult

    nc = bacc.Bacc("TRN2", target_bir_lowering=False, debug=False, num_devices=NCORES)

    hidden_t = nc.dram_tensor("hidden_t", [D, S], bf16, kind="ExternalInput")
    w_qkv = nc.dram_tensor("w_qkv", [D, 3 * 128], bf16, kind="ExternalInput")
    b_qkv = nc.dram_tensor("b_qkv", [128, 3], f32, kind="ExternalInput")
    wpj = nc.dram_tensor("wpj", [128, D], bf16, kind="ExternalInput")
    maskd = nc.dram_tensor("maskd", [128, 128], bf16, kind="ExternalInput")
    out_s = nc.dram_tensor("out_s", [S, D], bf16, kind="ExternalOutput")

    with tile.TileContext(nc) as tc:
        with (
            tc.tile_pool(name="persist", bufs=1) as pp,
            tc.tile_pool(name="epool", bufs=18) as ep,
            tc.tile_pool(name="apool", bufs=8) as apo,
            tc.tile_pool(name="tpool", bufs=3) as tpo,
            tc.tile_pool(name="opool", bufs=3) as op,
            tc.tile_pool(name="rpool", bufs=3) as rp,
            tc.tile_pool(name="ps_sc", bufs=2, space="PSUM") as ps_sc,
            tc.tile_pool(name="ps_pv", bufs=2, space="PSUM") as ps_pv,
            tc.tile_pool(name="ps_cp", bufs=2, space="PSUM") as ps_cp,
        ):
            # ---- small weights first, then all of hidden ----
            bias = pp.tile([128, 3], f32, tag="bias", name="bias")
            nc.sync.dma_start(bias[:], b_qkv.ap())
            wq = []
            for d in range(DC):
                w = pp.tile([128, 3 * 128], bf16, tag=f"wq{d}", name=f"wq{d}")
                nc.sync.dma_start(w[:], w_qkv.ap()[d * 128:(d + 1) * 128, :])
                wq.append(w)
            wpj_t = pp.tile([128, D], bf16, tag="wpj", name="wpj")
            nc.sync.dma_start(wpj_t[:], wpj.ap())
            maskt = pp.tile([128, 128], bf16, tag="mask", name="maskt")
            nc.sync.dma_start(maskt[:], maskd.ap())

            hT = [pp.tile([128, S], bf16, tag=f"hT{d}", name=f"hT{d}") for d in range(DC)]
            for d in range(DC):
                nc.sync.dma_start(hT[d][:], hidden_t.ap()[d * 128:(d + 1) * 128, :])

            # qkv m-block destinations (transposed layouts, [rows, seq])
            qcT = pp.tile([128, S], bf16, tag="qcT", name="qcT")
            kcT = pp.tile([128, S], bf16, tag="kcT", name="kcT")
            vcT = pp.tile([128, S], bf16, tag="vcT", name="vcT")
            # head 3 lives at partition 96; PE wants operand base in {0,32,64}
            qc3 = pp.tile([64, S], bf16, tag="qc3", name="qc3")
            kc3 = pp.tile([64, S], bf16, tag="kc3", name="kc3")

            # v_comp per k-tile: [kpos 128, 4 x (32 attn | 1 ones)]
            vco = pp.tile([128, NKT * HPC * VW], bf16, tag="vco", name="vco")
            nc.vector.memset(
                vco[:].rearrange("p (t w) -> p t w", w=VW)[:, :, C:C + 1], 1.0
            )

            def emit_qkv_chunk(sb, mb, psq):
                """One m-block (8 accum matmuls) + its PSUM->SBUF copy."""
                sl = slice(sb * SB, (sb + 1) * SB)
                half = mb % 2 if mb < 2 else 0
                for d in range(DC):
                    nc.tensor.matmul(
                        psq[:, half * SB:(half + 1) * SB],
                        wq[d][:, mb * 128:(mb + 1) * 128],
                        hT[d][:, sl],
                        start=(d == 0),
                        stop=(d == DC - 1),
                    )
                dest = (qcT, kcT, vcT)[mb]
                nc.vector.tensor_scalar_add(
                    out=dest[:, sl],
                    in0=psq[:, half * SB:(half + 1) * SB],
                    scalar1=bias[:, mb:mb + 1],
                )
                if mb == 1:
                    nc.sync.dma_start(qc3[32:64, sl], qcT[96:128, sl])
                    nc.sync.dma_start(kc3[32:64, sl], kcT[96:128, sl])
                if mb == 2:
                    for kt in range(4 * sb, 4 * sb + 4):
                        vtmp = tpo.tile([128, 128], bf16, tag="attnT", name="vtmp")
                        nc.sync.dma_start_transpose(
                            vtmp[:], vcT[:, kt * 128:(kt + 1) * 128]
                        )
                        nc.gpsimd.tensor_copy(
                            vco[:, kt * HPC * VW:(kt + 1) * HPC * VW]
                            .rearrange("p (h w) -> p h w", h=HPC)[:, :, 0:C],
                            vtmp[:].rearrange("p (h c) -> p h c", h=HPC),
                        )

            def emit_qkv(sb):
                psA = ps_sc.tile([128, 2 * SB], f32, tag="psS", name="psA")
                emit_qkv_chunk(sb, 0, psA)
                emit_qkv_chunk(sb, 1, psA)
                psB = ps_sc.tile([128, 2 * SB], f32, tag="psS", name="psB")
                emit_qkv_chunk(sb, 2, psB)

            # ---- per-sb attention state ----
            st = {}

            def emit_scores(sb, h, batch):
                if h == 3:
                    ksrc, qsrc, hsl = kc3, qc3, slice(32, 64)
                else:
                    ksrc, qsrc, hsl = kcT, qcT, slice(32 * h, 32 * h + 32)
                psS = ps_sc.tile([128, 2 * SB], f32, tag="psS", name="psS")
                e = ep.tile([128, 2 * SB], bf16, tag="E", name="e")
                first_c0 = None
                for i, kb in enumerate(batch):
                    r = kb - 4 * sb
                    c0 = max(r, 0) * 128
                    if first_c0 is None:
                        first_c0 = c0
                    nc.tensor.matmul(
                        psS[:, i * SB + c0:(i + 1) * SB],
                        ksrc[hsl, kb * 128:(kb + 1) * 128],
                        qsrc[hsl, sb * SB + c0:(sb + 1) * SB],
                    )
                    st[(sb, "e", h, kb)] = (e, i * SB, c0)
                last = len(batch) * SB
                nc.scalar.activation(
                    e[:, first_c0:last], psS[:, first_c0:last], Exp
                )
                for i, kb in enumerate(batch):
                    r = kb - 4 * sb
                    if r >= 0:
                        c0 = r * 128
                        nc.gpsimd.tensor_tensor(
                            e[:, i * SB + c0:i * SB + c0 + 128],
                            e[:, i * SB + c0:i * SB + c0 + 128],
                            maskt[:], mult,
                        )

            def pv_pack(sb, h):
                """All 4 query-block PV bursts for head h, plus their norms.

                One [128, 4x33] psum tile per head; each (h, qb) accumulation
                group opens and closes consecutively, so the bank never holds
                two open groups."""
                pvh = ps_pv.tile([128, HPC * VW], f32, tag="psP", name="psP")
                for qb in range(4 * sb, 4 * sb + 4):
                    col = (qb % 4) * VW
                    for kb in range(qb + 1):
                        e, off, c0 = st[(sb, "e", h, kb)]
                        qo = (qb % 4) * 128
                        nc.tensor.matmul(
                            pvh[:, col:col + VW],
                            e[:, off + qo:off + qo + 128],
                            vco[:, kb * HPC * VW + h * VW:kb * HPC * VW + (h + 1) * VW],
                            start=(kb == 0),
                            stop=(kb == qb),
                        )
                rc = rp.tile([128, HPC], f32, tag="rc", name="rc")
                nc.vector.reciprocal(
                    rc[:].unsqueeze(2),
                    pvh[:].rearrange("p (h w) -> p h w", h=HPC)[:, :, C:C + 1],
                )
                for qb in range(4 * sb, 4 * sb + 4):
                    col = (qb % 4) * VW
                    if h == 0:
                        a = apo.tile([128, 128], bf16, tag="attn", name="attn")
                        st[(sb, "a", qb)] = a
                    a = st[(sb, "a", qb)]
                    nc.vector.tensor_scalar_mul(
                        out=a[:, h * C:(h + 1) * C],
                        in0=pvh[:, col:col + C],
                        scalar1=rc[:, qb % 4:qb % 4 + 1],
                    )

            def finish_sb(sb):
                """Head-3 PV + transposes + c_proj + output DMA for sb."""
                pv_pack(sb, 3)
                for qb in range(4 * sb, 4 * sb + 4):
                    a = st.pop((sb, "a", qb))
                    aT = tpo.tile([128, 128], bf16, tag="attnT", name="attnT")
                    nc.sync.dma_start_transpose(aT[:], a[:])
                    stage = op.tile([128, 2 * SB], bf16, tag="stage", name="stage")
                    for half in range(2):
                        pso = ps_cp.tile([128, SB], f32, tag="psO", name="psO")
                        nc.tensor.matmul(
                            pso[:],
                            aT[:],
                            wpj_t[:, half * SB:(half + 1) * SB],
                        )
                        nc.vector.tensor_copy(
                            stage[:, half * SB:(half + 1) * SB], pso[:]
                        )
                    nc.sync.dma_start(
                        out_s.ap()[qb * 128:(qb + 1) * 128, :], stage[:]
                    )
                for h in range(HPC):
                    for kb in range(4 * sb + 4):
                        st.pop((sb, "e", h, kb), None)

            def emit_qkv_step(sb1, idx):
                if idx == 0:
                    st["psq"] = ps_sc.tile([128, 2 * SB], f32, tag="psS", name="psA")
                    emit_qkv_chunk(sb1, 0, st["psq"])
                elif idx == 1:
                    emit_qkv_chunk(sb1, 1, st["psq"])
                else:
                    psB = ps_sc.tile([128, 2 * SB], f32, tag="psS", name="psB")
                    emit_qkv_chunk(sb1, 2, psB)

            emit_qkv(0)
            for sb in range(NSB):
                nkb = 4 * sb + 4
                batches = [
                    list(range(bb, min(bb + 2, nkb))) for bb in range(0, nkb, 2)
                ]
                qkv_done = 0
                for h in range(HPC):
                    for b, batch in enumerate(batches):
                        emit_scores(sb, h, batch)
                        if b == 1:
                            if h == 0:
                                if sb > 0:
                                    finish_sb(sb - 1)
                            else:
                                pv_pack(sb, h - 1)
                        if h == 3 and sb < NSB - 1 and qkv_done < 3:
                            emit_qkv_step(sb + 1, qkv_done)
                            qkv_done += 1
                if sb < NSB - 1:
                    while qkv_done < 3:
                        emit_qkv_step(sb + 1, qkv_done)
                        qkv_done += 1
            finish_sb(NSB - 1)

    nc.compile()
    return nc


def _prep_inputs(hidden_states, w_attn, b_attn, wk_c, wv_c, wk_d, wv_d, w_proj):
    """Per-core input maps: fold all compressors/decompressors into weights.

      w_q' = w_q @ wk_d^T / sqrt(hd)   [D, C]  (scores contract over C)
      w_k' = w_k @ wk_c                [D, C]
      w_v' = w_v @ wv_c                [D, C]  (v-compress folded into qkv)
      w_p'_h = wv_d_h @ w_proj_h       [C, D]  (c_proj contracts over 4h x C)
    """
    f8 = np.float64
    hidden_T = [np.ascontiguousarray(hidden_states[b].T).astype(BF16) for b in range(B)]
    scale = 1.0 / np.sqrt(hd)
    wq_h = lambda h: (w_attn[:, h * hd:(h + 1) * hd].astype(f8)
                      @ wk_d[h].astype(f8).T * scale).astype(np.float32)
    bq_h = lambda h: (b_attn[h * hd:(h + 1) * hd].astype(f8)
                      @ wk_d[h].astype(f8).T * scale).astype(np.float32)
    wk_h = lambda h: (w_attn[:, D + h * hd:D + (h + 1) * hd].astype(f8)
                      @ wk_c[h].astype(f8)).astype(np.float32)
    bk_h = lambda h: (b_attn[D + h * hd:D + (h + 1) * hd].astype(f8)
                      @ wk_c[h].astype(f8)).astype(np.float32)
    wv_h = lambda h: (w_attn[:, 2 * D + h * hd:2 * D + (h + 1) * hd].astype(f8)
                      @ wv_c[h].astype(f8)).astype(np.float32)
    bv_h = lambda h: (b_attn[2 * D + h * hd:2 * D + (h + 1) * hd].astype(f8)
                      @ wv_c[h].astype(f8)).astype(np.float32)
    wpj_h = lambda h: (wv_d[h].astype(f8)
                       @ w_proj[h * hd:(h + 1) * hd, :].astype(f8)).astype(np.float32)

    k = np.arange(128).reshape(128, 1)
    j = np.arange(128).reshape(1, 128)
    mask = (k <= j).astype(BF16)

    in_maps = []
    for c in range(NCORES):
        b = c // 4
        hs = list(range((c % 4) * HPC, (c % 4) * HPC + HPC))
        # m-blocks: [q'0..3], [kc0..3], [vc0..3]
        cols = ([wq_h(h) for h in hs] + [wk_h(h) for h in hs]
                + [wv_h(h) for h in hs])
        w_qkv_l = np.concatenate(cols, axis=1).astype(BF16)        # [1024, 384]
        bcols = ([bq_h(h) for h in hs] + [bk_h(h) for h in hs]
                 + [bv_h(h) for h in hs])
        b_qkv_l = (np.concatenate(bcols).astype(np.float32)
                   .reshape(3, 128).T.copy())                      # [128, 3]
        # w_proj' stacked over the core's 4 heads: [4h x 32, D]
        wpj_l = np.concatenate([wpj_h(h) for h in hs], axis=0).astype(BF16)
        in_maps.append(
            {
                "hidden_t": hidden_T[b],
                "w_qkv": w_qkv_l,
                "b_qkv": b_qkv_l,
                "wpj": wpj_l,
                "maskd": np.ascontiguousarray(mask),
            }
        )
    return in_maps


def kernel(
    hidden_states,
    w_attn,
    b_attn,
    w_proj,
    b_proj,
    wk_c,
    wv_c,
    wk_d,
    wv_d,
    _trace=False,
):
    from concourse.bass_utils import run_bass_kernel_spmd

    if "nc" not in _cache:
        _cache["nc"] = _build()
    nc = _cache["nc"]

    in_maps = _prep_inputs(
        np.asarray(hidden_states),
        np.asarray(w_attn),
        np.asarray(b_attn),
        np.asarray(wk_c),
        np.asarray(wv_c),
        np.asarray(wk_d),
        np.asarray(wv_d),
        np.asarray(w_proj),
    )
    res = run_bass_kernel_spmd(
        nc, in_maps, core_ids=list(range(NCORES)), trace=_trace
    )
    out = np.empty((B, S, D), np.float32)
    for b in range(B):
        acc = np.zeros((S, D), np.float32)
        for c in range(4 * b, 4 * b + 4):
            acc += res.results[c]["out_s"].astype(np.float32)
        out[b] = acc + np.asarray(b_proj, np.float32)
    if _trace:
        _cache["last_exec_time_ns"] = res.exec_time_ns
        _cache["last_results"] = res
    return out


# revision 13
# speedup vs baseline: 1.3705x; 1.0416x over previous
"""Compressed-KV GPT-2 attention block on 8 TRN2 NeuronCores.

Sharding: batch x head-group. Core c: batch b = c//4, heads 4*(c%4)..4*(c%4)+4.

Structure (v4): attention runs in the compressed C-space (C=32 < hd=64);
all decompressors fold into host-side weights, and the v-compressor folds
into the qkv projection:
  w_q' = w_q wk_d^T/8 [D,32/head]   w_k' = w_k wk_c [D,32/head]
  w_v' = w_v wv_c    [D,32/head]    w_p' = wv_d_h w_proj_h [32,D]
Device pipeline per core (bf16 matmuls -> fp32 PSUM):
  qkv^T    = w' chunks @ hidden^T      (3 m-blocks: q'(4hx32) | kc | vc)
  vco      = DMA-transpose(vc^T k-tile) -> [kpos, 4x(32|ones)] SBUF
  S^T      = kc-slice^T @ q'  (K=32 per head) -> 2-bank batched exp (Scalar)
             -> diagonal band tiles masked on GpSimd
  attn[q,c]= E-tile^T(stationary) @ vco-slice, per-(head,qb) bursts so each
             PSUM bank has one open accumulation group (M=128 queries, N=33:
             col 32 = ones = softmax denom per query partition)
  norm     = one DVE divide per (head, qb): psum/den -> attn_all bf16
  attn^T   = DMA-transpose -> c_proj: one K=128 contraction over all heads
  out[q,d] 2 x 512-col matmuls per 128-query block, DMA'd to out_s [S, D]
Schedule: PV bursts of head h-1 interleave into head h's score batches;
qkv(sb+1) m-block chunks interleave into head 3's batches; the previous
sb's PV(h3)+cproj interleave into the next sb's head-0 batches. This keeps
the Scalar engine (exp, the ~73us roofline here) continuously fed.
"""

import sys

if "/opt/trn_rl_repo" not in sys.path:
    sys.path.insert(0, "/opt/trn_rl_repo")

import numpy as np
import ml_dtypes

BF16 = ml_dtypes.bfloat16

B, S, D = 2, 2048, 1024
H, hd, C = 16, 64, 32
NCORES = 8
HPC = 4            # heads per core
SB = 512           # q block per score tile / PSUM bank cols
NSB = S // SB      # 4 seq blocks of 512
NKT = S // 128     # 16 key tiles of 128
DC = D // 128      # 8 contraction chunks for qkv
VW = C + 1         # attn + den column width per head slot

_cache = {}


def _build():
    import concourse.bacc as bacc
    import concourse.tile as tile
    import concourse.mybir as mybir

    dt = mybir.dt
    f32, bf16 = dt.float32, dt.bfloat16
    Exp = mybir.ActivationFunctionType.Exp
    mult = mybir.AluOpType.mult

    nc = bacc.Bacc("TRN2", target_bir_lowering=False, debug=False, num_devices=NCORES)

    hidden_t = nc.dram_tensor("hidden_t", [D, S], bf16, kind="ExternalInput")
    w_qkv = nc.dram_tensor("w_qkv", [D, 3 * 128], bf16, kind="ExternalInput")
    b_qkv = nc.dram_tensor("b_qkv", [128, 3], f32, kind="ExternalInput")
    wpj = nc.dram_tensor("wpj", [128, D], bf16, kind="ExternalInput")
    maskd = nc.dram_tensor("maskd", [128, 128], bf16, kind="ExternalInput")
    out_s = nc.dram_tensor("out_s", [S, D], bf16, kind="ExternalOutput")

    with tile.TileContext(nc) as tc:
        with (
            tc.tile_pool(name="persist", bufs=1) as pp,
            tc.tile_pool(name="epool", bufs=18) as ep,
            tc.tile_pool(name="apool", bufs=8) as apo,
            tc.tile_pool(name="tpool", bufs=3) as tpo,
            tc.tile_pool(name="opool", bufs=3) as op,
            tc.tile_pool(name="rpool", bufs=3) as rp,
            tc.tile_pool(name="ps_sc", bufs=3, space="PSUM") as ps_sc,
            tc.tile_pool(name="ps_pv", bufs=1, space="PSUM") as ps_pv,
            tc.tile_pool(name="ps_cp", bufs=1, space="PSUM") as ps_cp,
        ):
            # ---- small weights first, then all of hidden ----
            bias = pp.tile([128, 3], f32, tag="bias", name="bias")
            nc.sync.dma_start(bias[:], b_qkv.ap())
            wq = []
            for d in range(DC):
                w = pp.tile([128, 3 * 128], bf16, tag=f"wq{d}", name=f"wq{d}")
                nc.sync.dma_start(w[:], w_qkv.ap()[d * 128:(d + 1) * 128, :])
                wq.append(w)
            wpj_t = pp.tile([128, D], bf16, tag="wpj", name="wpj")
            nc.sync.dma_start(wpj_t[:], wpj.ap())
            maskt = pp.tile([128, 128], bf16, tag="mask", name="maskt")
            nc.sync.dma_start(maskt[:], maskd.ap())

            hT = [pp.tile([128, S], bf16, tag=f"hT{d}", name=f"hT{d}") for d in range(DC)]
            for d in range(DC):
                nc.sync.dma_start(hT[d][:], hidden_t.ap()[d * 128:(d + 1) * 128, :])

            # qkv m-block destinations (transposed layouts, [rows, seq])
            qcT = pp.tile([128, S], bf16, tag="qcT", name="qcT")
            kcT = pp.tile([128, S], bf16, tag="kcT", name="kcT")
            vcT = pp.tile([128, S], bf16, tag="vcT", name="vcT")

            # v_comp per k-tile: [kpos 128, 4 x (32 attn | 1 ones)]
            vco = pp.tile([128, NKT * HPC * VW], bf16, tag="vco", name="vco")
            nc.vector.memset(
                vco[:].rearrange("p (t w) -> p t w", w=VW)[:, :, C:C + 1], 1.0
            )

            def emit_qkv_chunk(sb, mb, psq):
                """One m-block (8 accum matmuls) + its PSUM->SBUF copy."""
                sl = slice(sb * SB, (sb + 1) * SB)
                half = mb % 2 if mb < 2 else 0
                for d in range(DC):
                    nc.tensor.matmul(
                        psq[:, half * SB:(half + 1) * SB],
                        wq[d][:, mb * 128:(mb + 1) * 128],
                        hT[d][:, sl],
                        start=(d == 0),
                        stop=(d == DC - 1),
                    )
                dest = (qcT, kcT, vcT)[mb]
                nc.vector.tensor_scalar_add(
                    out=dest[:, sl],
                    in0=psq[:, half * SB:(half + 1) * SB],
                    scalar1=bias[:, mb:mb + 1],
                )
                if mb == 2:
                    for kt in range(4 * sb, 4 * sb + 4):
                        vtmp = tpo.tile([128, 128], bf16, tag="attnT", name="vtmp")
                        nc.sync.dma_start_transpose(
                            vtmp[:], vcT[:, kt * 128:(kt + 1) * 128]
                        )
                        nc.gpsimd.tensor_copy(
                            vco[:, kt * HPC * VW:(kt + 1) * HPC * VW]
                            .rearrange("p (h w) -> p h w", h=HPC)[:, :, 0:C],
                            vtmp[:].rearrange("p (h c) -> p h c", h=HPC),
                        )

            def emit_qkv(sb):
                psA = ps_sc.tile([128, 2 * SB], f32, tag="psS", name="psA")
                emit_qkv_chunk(sb, 0, psA)
                emit_qkv_chunk(sb, 1, psA)
                psB = ps_sc.tile([128, 2 * SB], f32, tag="psS", name="psB")
                emit_qkv_chunk(sb, 2, psB)

            # ---- per-sb attention state ----
            st = {}

            def emit_scores(sb, h, batch):
                ksrc, qsrc, hsl = kcT, qcT, slice(32 * h, 32 * h + 32)
                psS = ps_sc.tile([128, 2 * SB], f32, tag="psS", name="psS")
                e = ep.tile([128, 2 * SB], bf16, tag="E", name="e")
                first_c0 = None
                for i, kb in enumerate(batch):
                    r = kb - 4 * sb
                    c0 = max(r, 0) * 128
                    if first_c0 is None:
                        first_c0 = c0
                    nc.tensor.matmul(
                        psS[:, i * SB + c0:(i + 1) * SB],
                        ksrc[hsl, kb * 128:(kb + 1) * 128],
                        qsrc[hsl, sb * SB + c0:(sb + 1) * SB],
                        tile_position=(32 * h, 0),
                    )
                    st[(sb, "e", h, kb)] = (e, i * SB, c0)
                last = len(batch) * SB
                nc.scalar.activation(
                    e[:, first_c0:last], psS[:, first_c0:last], Exp
                )
                for i, kb in enumerate(batch):
                    r = kb - 4 * sb
                    if r >= 0:
                        c0 = r * 128
                        nc.gpsimd.tensor_tensor(
                            e[:, i * SB + c0:i * SB + c0 + 128],
                            e[:, i * SB + c0:i * SB + c0 + 128],
                            maskt[:], mult,
                        )

            def pv_pack(sb, h):
                """All 4 query-block PV bursts for head h, plus their norms.

                One [128, 4x33] psum tile per head; each (h, qb) accumulation
                group opens and closes consecutively, so the bank never holds
                two open groups."""
                pvh = ps_pv.tile([128, HPC * VW], f32, tag="psP", name="psP")
                for qb in range(4 * sb, 4 * sb + 4):
                    col = (qb % 4) * VW
                    for kb in range(qb + 1):
                        e, off, c0 = st[(sb, "e", h, kb)]
                        qo = (qb % 4) * 128
                        nc.tensor.matmul(
                            pvh[:, col:col + VW],
                            e[:, off + qo:off + qo + 128],
                            vco[:, kb * HPC * VW + h * VW:kb * HPC * VW + (h + 1) * VW],
                            start=(kb == 0),
                            stop=(kb == qb),
                        )
                rc = rp.tile([128, HPC], f32, tag="rc", name="rc")
                nc.vector.reciprocal(
                    rc[:].unsqueeze(2),
                    pvh[:].rearrange("p (h w) -> p h w", h=HPC)[:, :, C:C + 1],
                )
                for qb in range(4 * sb, 4 * sb + 4):
                    col = (qb % 4) * VW
                    if h == 0:
                        a = apo.tile([128, 128], bf16, tag="attn", name="attn")
                        st[(sb, "a", qb)] = a
                    a = st[(sb, "a", qb)]
                    nc.vector.tensor_scalar_mul(
                        out=a[:, h * C:(h + 1) * C],
                        in0=pvh[:, col:col + C],
                        scalar1=rc[:, qb % 4:qb % 4 + 1],
                    )

            def finish_sb(sb):
                """Head-3 PV + transposes + c_proj + output DMA for sb."""
                pv_pack(sb, 3)
                for qb in range(4 * sb, 4 * sb + 4):
                    a = st.pop((sb, "a", qb))
                    aT = tpo.tile([128, 128], bf16, tag="attnT", name="attnT")
                    nc.sync.dma_start_transpose(aT[:], a[:])
                    stage = op.tile([128, 2 * SB], bf16, tag="stage", name="stage")
                    for half in range(2):
                        pso = ps_cp.tile([128, SB], f32, tag="psO", name="psO")
                        nc.tensor.matmul(
                            pso[:],
                            aT[:],
                            wpj_t[:, half * SB:(half + 1) * SB],
                        )
                        nc.vector.tensor_copy(
                            stage[:, half * SB:(half + 1) * SB], pso[:]
                        )
                    nc.sync.dma_start(
                        out_s.ap()[qb * 128:(qb + 1) * 128, :], stage[:]
                    )
                for h in range(HPC):
                    for kb in range(4 * sb + 4):
                        st.pop((sb, "e", h, kb), None)

            def emit_qkv_step(sb1, idx):
                if idx == 0:
                    st["psq"] = ps_sc.tile([128, 2 * SB], f32, tag="psS", name="psA")
                    emit_qkv_chunk(sb1, 0, st["psq"])
                elif idx == 1:
                    emit_qkv_chunk(sb1, 1, st["psq"])
                else:
                    psB = ps_sc.tile([128, 2 * SB], f32, tag="psS", name="psB")
                    emit_qkv_chunk(sb1, 2, psB)

            emit_qkv(0)
            for sb in range(NSB):
                nkb = 4 * sb + 4
                batches = [
                    list(range(bb, min(bb + 2, nkb))) for bb in range(0, nkb, 2)
                ]
                qkv_done = 0
                for h in range(HPC):
                    for b, batch in enumerate(batches):
                        emit_scores(sb, h, batch)
                        if b == 1:
                            if h == 0:
                                if sb > 0:
                                    finish_sb(sb - 1)
                            else:
                                pv_pack(sb, h - 1)
                        if h == 3 and sb < NSB - 1 and qkv_done < 3:
                            emit_qkv_step(sb + 1, qkv_done)
                            qkv_done += 1
                if sb < NSB - 1:
                    while qkv_done < 3:
                        emit_qkv_step(sb + 1, qkv_done)
                        qkv_done += 1
            finish_sb(NSB - 1)

    nc.compile()
    return nc


def _prep_inputs(hidden_states, w_attn, b_attn, wk_c, wv_c, wk_d, wv_d, w_proj):
    """Per-core input maps: fold all compressors/decompressors into weights.

      w_q' = w_q @ wk_d^T / sqrt(hd)   [D, C]  (scores contract over C)
      w_k' = w_k @ wk_c                [D, C]
      w_v' = w_v @ wv_c                [D, C]  (v-compress folded into qkv)
      w_p'_h = wv_d_h @ w_proj_h       [C, D]  (c_proj contracts over 4h x C)
    """
    f8 = np.float64
    hidden_T = [np.ascontiguousarray(hidden_states[b].T).astype(BF16) for b in range(B)]
    scale = 1.0 / np.sqrt(hd)
    wq_h = lambda h: (w_attn[:, h * hd:(h + 1) * hd].astype(f8)
                      @ wk_d[h].astype(f8).T * scale).astype(np.float32)
    bq_h = lambda h: (b_attn[h * hd:(h + 1) * hd].astype(f8)
                      @ wk_d[h].astype(f8).T * scale).astype(np.float32)
    wk_h = lambda h: (w_attn[:, D + h * hd:D + (h + 1) * hd].astype(f8)
                      @ wk_c[h].astype(f8)).astype(np.float32)
    bk_h = lambda h: (b_attn[D + h * hd:D + (h + 1) * hd].astype(f8)
                      @ wk_c[h].astype(f8)).astype(np.float32)
    wv_h = lambda h: (w_attn[:, 2 * D + h * hd:2 * D + (h + 1) * hd].astype(f8)
                      @ wv_c[h].astype(f8)).astype(np.float32)
    bv_h = lambda h: (b_attn[2 * D + h * hd:2 * D + (h + 1) * hd].astype(f8)
                      @ wv_c[h].astype(f8)).astype(np.float32)
    wpj_h = lambda h: (wv_d[h].astype(f8)
                       @ w_proj[h * hd:(h + 1) * hd, :].astype(f8)).astype(np.float32)

    k = np.arange(128).reshape(128, 1)
    j = np.arange(128).reshape(1, 128)
    mask = (k <= j).astype(BF16)

    in_maps = []
    for c in range(NCORES):
        b = c // 4
        hs = list(range((c % 4) * HPC, (c % 4) * HPC + HPC))
        # m-blocks: [q'0..3], [kc0..3], [vc0..3]
        cols = ([wq_h(h) for h in hs] + [wk_h(h) for h in hs]
                + [wv_h(h) for h in hs])
        w_qkv_l = np.concatenate(cols, axis=1).astype(BF16)        # [1024, 384]
        bcols = ([bq_h(h) for h in hs] + [bk_h(h) for h in hs]
                 + [bv_h(h) for h in hs])
        b_qkv_l = (np.concatenate(bcols).astype(np.float32)
                   .reshape(3, 128).T.copy())                      # [128, 3]
        # w_proj' stacked over the core's 4 heads: [4h x 32, D]
        wpj_l = np.concatenate([wpj_h(h) for h in hs], axis=0).astype(BF16)
        in_maps.append(
            {
                "hidden_t": hidden_T[b],
                "w_qkv": w_qkv_l,
                "b_qkv": b_qkv_l,
                "wpj": wpj_l,
                "maskd": np.ascontiguousarray(mask),
            }
        )
    return in_maps


def kernel(
    hidden_states,
    w_attn,
    b_attn,
    w_proj,
    b_proj,
    wk_c,
    wv_c,
    wk_d,
    wv_d,
    _trace=False,
):
    from concourse.bass_utils import run_bass_kernel_spmd

    if "nc" not in _cache:
        _cache["nc"] = _build()
    nc = _cache["nc"]

    in_maps = _prep_inputs(
        np.asarray(hidden_states),
        np.asarray(w_attn),
        np.asarray(b_attn),
        np.asarray(wk_c),
        np.asarray(wv_c),
        np.asarray(wk_d),
        np.asarray(wv_d),
        np.asarray(w_proj),
    )
    res = run_bass_kernel_spmd(
        nc, in_maps, core_ids=list(range(NCORES)), trace=_trace
    )
    out = np.empty((B, S, D), np.float32)
    for b in range(B):
        acc = np.zeros((S, D), np.float32)
        for c in range(4 * b, 4 * b + 4):
            acc += res.results[c]["out_s"].astype(np.float32)
        out[b] = acc + np.asarray(b_proj, np.float32)
    if _trace:
        _cache["last_exec_time_ns"] = res.exec_time_ns
        _cache["last_results"] = res
    return out
